# revision 23
# baseline (speedup 1.0000x reference)
"""Multi-head attention (B=2, S=2048, D=1024, H=16) on one TRN2 chip (8 cores).

Sharding (Megatron-style): DP=2 over batch x TP=4 over heads.
Core c (c = 0..7): batch g = c//4, heads [4r, 4r+4) where r = c%4.

Per-core pipeline (all matmul operands bf16, host-cast; accumulation fp32):
  - inputs arrive pre-transposed AND pre-swizzled so every device DMA is a
    fully contiguous HWDGE transfer on the sync queue (no SWDGE casts).
  - Q^T/K^T [256, S] and V [S, 256] projections (fp32 accum in PSUM).
  - attention per head in "scores transposed" layout (scores^T[k, q]):
    softmax without max-subtraction (logits are O(1)), denominator obtained
    free via a ones column appended to V.  exp is batched 2 heads per
    ACTIVATE ([128, 2x512] PSUM tiles) to amortize the ~352-cycle ACT
    startup; the softmax reciprocal runs on the vector engine (no scalar
    table switches -- scalar engine runs Exp only).
  - scores for key-tile kt+1 are emitted before the ctx matmul of kt so the
    PE never sits behind the scalar engine's exp.
  - output projection per 128-row slice; each 256-row half-chunk is
    ReduceScattered(add) over the 4-core TP group directly into the output
    tensor.  The gpsimd queue carries ONLY the collective triggers and tiny
    partition broadcasts, so collectives never stall compute.
Host assembles the 8 cores' shard chunks and adds the output bias.

Mask handling (kernel inspects the mask input on the host):
  - canonical causal mask -> fast path: upper-triangle key blocks skipped,
    diagonal blocks get an on-device generated additive mask.
  - all-zeros mask -> dense path, no mask applied.
  - anything else -> generic path: mask^T * sqrt(DH) streamed from DRAM
    and added to every score tile (matches exp(s*scale + m) exactly).
"""

from contextlib import ExitStack

import ml_dtypes
import numpy as np

import concourse.bacc as bacc
import concourse.mybir as mybir
import concourse.tile as tile
from concourse.bass_utils import run_bass_kernel_spmd

F32 = mybir.dt.float32
BF16 = mybir.dt.bfloat16
AF = mybir.ActivationFunctionType

H = 16
D = 1024
B = 2
S = 2048
DH = 64
N_CORES = 8
DP = 2                      # data-parallel groups (over batch)
TP = N_CORES // DP          # tensor-parallel cores per group
HPC = H // TP               # heads per core = 4
DHH = HPC * DH              # 256 features per core
NEG = -1e9

P = 128                     # partitions
FD = 512                    # matmul moving free dim (one PSUM bank fp32)


def _emit(tc, io, mask_mode, s, with_bias=True):
    with ExitStack() as _stk:
        _emit_inner(_stk, tc, io, mask_mode, s, with_bias)


def _emit_inner(stk, tc, io, mask_mode, s, with_bias):
    nc = tc.nc
    NQ = s // FD            # query chunks
    NK = s // P             # key tiles
    ND = D // P             # d-model tiles = 8
    NH2 = HPC // 2          # head pairs = 2
    SPC = FD // P           # seq-tiles per chunk = 4

    const = stk.enter_context(tc.tile_pool(name="const", bufs=1))
    persist = stk.enter_context(tc.tile_pool(name="persist", bufs=1))
    dram = stk.enter_context(tc.tile_pool(name="dram", bufs=1, space="DRAM"))

    # ---- constants -------------------------------------------------------
    onescol = const.tile([P, 1], F32)
    nc.vector.memset(onescol, 1.0)
    if with_bias:
        ones_f32 = const.tile([1, FD], F32)
        nc.vector.memset(ones_f32, 1.0)
        ones = const.tile([1, FD], BF16)
        nc.vector.tensor_copy(ones, ones_f32)

    if mask_mode == "causal":
        # triangular mask tile: allowed (0) iff qf - kp >= 0 else NEG
        dmask = const.tile([P, 1, P], F32)
        nc.gpsimd.memset(dmask, 0.0)
        nc.gpsimd.affine_select(
            out=dmask[:, 0, :],
            in_=dmask[:, 0, :],
            compare_op=mybir.AluOpType.is_ge,
            fill=NEG,
            base=0,
            pattern=[[1, P]],
            channel_multiplier=-1,
        )

    # ---- weights / biases (host pre-swizzled, bf16, contiguous DMA) -----
    # load weights in dt-halves so the first projection matmuls can start
    # before the whole tensor has landed
    HD = ND // 2
    w_sb = {}
    for name in ("wq", "wk", "wv"):
        w_sb[name] = persist.tile([P, ND, DHH], BF16, name=f"w_{name}")
        for h in range(2):
            nc.sync.dma_start(w_sb[name][:, h * HD:(h + 1) * HD, :],
                              io[name][:, h * HD:(h + 1) * HD, :])
    wo_sb = persist.tile([P, DHH // P, D], BF16)
    nc.sync.dma_start(wo_sb, io["wo"])

    b_sb = {}
    if with_bias:
        for name in ("bq", "bk", "bv"):
            b_sb[name] = const.tile([1, DHH], BF16, name=f"b_{name}")
            nc.sync.dma_start(b_sb[name], io[name])

    # ---- persistent activations: one tile per seq-chunk -----------------
    qT = [persist.tile([P, NH2, FD], BF16, name=f"qT{i}") for i in range(NQ)]
    kT = [persist.tile([P, NH2, FD], BF16, name=f"kT{i}") for i in range(NQ)]
    v_c = [persist.tile([P, SPC, HPC, DH + 1], BF16, name=f"v{i}")
           for i in range(NQ)]
    for i in range(NQ):                     # fill the ones columns
        nc.vector.tensor_copy(
            v_c[i][:, :, :, DH:DH + 1], onescol.to_broadcast((P, SPC, HPC, 1))
        )
    ctxT = [persist.tile([P, NH2, FD], BF16, name=f"ctxT{i}")
            for i in range(NQ)]

    scale = 1.0 / float(np.sqrt(DH))
    NQ_ = s // FD
    # one RS piece per chunk (1MB -- CC per-op cost is ~8us fixed + ~12us/MB
    # so fewer, larger ops win), EXCEPT the last chunk which is split in two
    # so the only RS exposed after compute ends is half-sized.
    pieces_of = lambda qc: 2 if qc == NQ_ - 1 else 1
    partial = {}
    for qc in range(NQ_):
        for p in range(pieces_of(qc)):
            partial[(qc, p)] = dram.tile(
                [FD // pieces_of(qc), D], BF16, name=f"partial_{qc}_{p}"
            )
    # single contiguous shard tensor: the RS ops write disjoint row slices
    # and ONE final DMA ships it to the output -- per-RS out-DMAs would each
    # gate on an RS completion and head-of-line block their queue.
    shard_all = dram.tile([s // TP, D], BF16, name="shard_all")
    groups = [list(range(g * TP, (g + 1) * TP)) for g in range(DP)]

    with (
        tc.tile_pool(name="xt", bufs=6) as xt_pool,
        tc.tile_pool(name="mm_ps", bufs=2, space="PSUM") as mm_ps_pool,
        tc.tile_pool(name="ctx_ps", bufs=4, space="PSUM") as ctx_ps_pool,
        tc.tile_pool(name="pt", bufs=4) as pt_pool,
        tc.tile_pool(name="mload", bufs=3) as mload_pool,
        tc.tile_pool(name="small", bufs=4) as small_pool,
        tc.tile_pool(name="bc_sb", bufs=4) as bc_sb_pool,
        tc.tile_pool(name="out_sb", bufs=3) as out_sb_pool,
    ):
        def project_chunk(sc):
            for tname, wname, bname, dstT in (
                ("xk", "wk", "bk", kT),
                ("xq", "wq", "bq", qT),
                ("xv", "wv", "bv", None),
            ):
                xt_c = xt_pool.tile([P, ND, FD], BF16, tag="xt",
                                    name=f"xt_{tname}_{sc}")
                for h in range(2):
                    nc.sync.dma_start(xt_c[:, h * HD:(h + 1) * HD, :],
                                      io[tname][sc][:, h * HD:(h + 1) * HD, :])
                if dstT is not None:
                    qps = mm_ps_pool.tile([P, NH2, FD], F32, tag="mm",
                                          name=f"ps_{tname}_{sc}")
                    for mt in range(NH2):
                        for dt in range(ND):
                            nc.tensor.matmul(
                                qps[:, mt, :],
                                w_sb[wname][:, dt, mt * P:(mt + 1) * P],
                                xt_c[:, dt, :],
                                start=(dt == 0),
                                stop=(not with_bias and dt == ND - 1),
                            )
                        if with_bias:
                            nc.tensor.matmul(  # + bias (ones-row augment)
                                qps[:, mt, :],
                                b_sb[bname][0:1, mt * P:(mt + 1) * P],
                                ones[0:1, :],
                                start=False,
                                stop=True,
                            )
                    nc.vector.tensor_copy(dstT[sc], qps)
                else:
                    vp = mm_ps_pool.tile([P, 2, 2, DHH], F32, tag="mm",
                                         name=f"ps_v_{sc}")
                    for st in range(SPC):
                        sl = vp[:, st // 2, st % 2, :]
                        for dt in range(ND):
                            nc.tensor.matmul(
                                sl,
                                xt_c[:, dt, st * P:(st + 1) * P],
                                w_sb[wname][:, dt, :],
                                start=(dt == 0),
                                stop=(not with_bias and dt == ND - 1),
                            )
                        if with_bias:
                            nc.tensor.matmul(
                                sl,
                                ones[0:1, 0:P],
                                b_sb[bname][0:1, :],
                                start=False,
                                stop=True,
                            )
                    nc.vector.tensor_copy(
                        v_c[sc][:, :, :, 0:DH],
                        vp.rearrange("p a b (h e) -> p (a b) h e", h=HPC),
                    )

        def attend_chunk(qc):
            nkt = (qc + 1) * SPC if mask_mode == "causal" else NK
            ctx = [
                ctx_ps_pool.tile([DH + 1, FD], F32, tag="ctx",
                                 name=f"ctx_{qc}_{hj}")
                for hj in range(4)
            ]

            def emit_ctx(kt, pts, q0, w):
                ksc, kti = kt // SPC, kt % SPC
                for hj in range(4):
                    hp, j = hj // 2, hj % 2
                    nc.tensor.matmul(
                        ctx[hj][:, q0:FD],
                        v_c[ksc][:, kti, hj, :],
                        pts[hp][:, j, 0:w],
                        start=(kt == 0),
                        stop=(kt == nkt - 1),
                    )

            pend = None
            for kt in range(nkt):
                ksc, kti = kt // SPC, kt % SPC
                dj = kt - qc * SPC
                mt_sb = None
                if mask_mode == "generic":
                    mt_sb = mload_pool.tile([P, 1, FD], F32, tag="ml")
                    nc.sync.dma_start(
                        mt_sb[:, 0, :],
                        io["maskT"][kt * P:(kt + 1) * P,
                                    qc * FD:(qc + 1) * FD],
                    )
                # causal diagonal tiles: queries below 128*dj see nothing
                # of this key tile -- compute only the valid q-range and
                # mask only the [P, P] sub-tile crossing the diagonal.
                # score tiles are origin-shifted: col f <-> query q0 + f.
                q0 = P * dj if (mask_mode == "causal" and dj > 0) else 0
                w = FD - q0
                pts = []
                for hp in range(NH2):
                    sp = mm_ps_pool.tile([P, NH2, FD], F32, tag="mm",
                                         name=f"sc_{qc}_{kt}_{hp}")
                    for j in range(2):
                        nc.tensor.matmul(
                            sp[:, j, 0:w],
                            kT[ksc][64 * j:64 * (j + 1), hp,
                                    kti * P:(kti + 1) * P],
                            qT[qc][64 * j:64 * (j + 1), hp, q0:FD],
                            start=True,
                            stop=True,
                        )
                    if mt_sb is not None:
                        nc.vector.tensor_add(
                            sp, sp, mt_sb.to_broadcast((P, NH2, FD))
                        )
                    elif mask_mode == "causal" and dj >= 0:
                        nc.vector.tensor_add(
                            sp[:, :, 0:P], sp[:, :, 0:P],
                            dmask.to_broadcast((P, NH2, P)),
                        )
                    pt = pt_pool.tile([P, NH2, FD], BF16, tag="pt")
                    nc.scalar.activation(pt[:, :, 0:w], sp[:, :, 0:w],
                                         AF.Exp, scale=scale)
                    pts.append(pt)
                # one-kt lookahead: emit ctx(kt-1) after scores(kt) so the
                # PE chews the previous tile while scalar runs this exp.
                if pend is not None:
                    emit_ctx(*pend)
                pend = (kt, pts, q0, w)
            emit_ctx(*pend)

            # normalize: rows 0..63 raw ctx^T, row 64 softmax denominator
            for hj in range(4):
                hp, j = hj // 2, hj % 2
                den = small_pool.tile([1, FD], F32, tag="den")
                nc.vector.tensor_copy(den, ctx[hj][DH:DH + 1, :])
                recip = small_pool.tile([1, FD], F32, tag="recip")
                # custom-DVE op: needs an SBUF input (PSUM reads diverge on
                # hardware); den >= exp(0) so no edge cases
                nc.vector.reciprocal_approx_fast(recip, den)
                bc = bc_sb_pool.tile([DH, FD], F32, tag="bc")
                nc.gpsimd.partition_broadcast(bc, recip)
                nc.vector.tensor_mul(
                    ctxT[qc][64 * j:64 * (j + 1), hp, :],
                    ctx[hj][0:DH, :],
                    bc,
                )

        def project_out_chunk(qc):
            pieces = pieces_of(qc)
            spp = SPC // pieces
            for p in range(pieces):
                for st2 in range(spp):
                    st = p * spp + st2
                    op = mm_ps_pool.tile([P, 2, FD], F32, tag="mm",
                                         name=f"op_{qc}_{st}")
                    for oc in range(2):
                        for hp in range(NH2):
                            nc.tensor.matmul(
                                op[:, oc, :],
                                ctxT[qc][:, hp, st * P:(st + 1) * P],
                                wo_sb[:, hp, oc * FD:(oc + 1) * FD],
                                start=(hp == 0),
                                stop=(hp == NH2 - 1),
                            )
                    ob = out_sb_pool.tile([P, D], BF16, tag="ob")
                    nc.vector.tensor_copy(ob,
                                          op.rearrange("p a f -> p (a f)"))
                    nc.sync.dma_start(
                        partial[(qc, p)][st2 * P:(st2 + 1) * P, :], ob
                    )
                rows = FD // pieces // TP
                off = (qc * FD + p * (FD // pieces)) // TP
                nc.gpsimd.collective_compute(
                    "ReduceScatter",
                    mybir.AluOpType.add,
                    replica_groups=groups,
                    ins=[partial[(qc, p)].opt()],
                    outs=[shard_all[off:off + rows, :].opt()],
                )

        if mask_mode == "causal":
            # stream: chunk qc's attention needs only K/V chunks <= qc.
            # project(c+1) is emitted before project_out(c) so the next
            # chunk's x^T DMAs enter the sync queue ahead of the partial
            # writes and prefetch during attend(c).
            project_chunk(0)
            for sc in range(NQ):
                attend_chunk(sc)
                if sc + 1 < NQ:
                    project_chunk(sc + 1)
                project_out_chunk(sc)
        else:
            for sc in range(NQ):
                project_chunk(sc)
            for qc in range(NQ):
                attend_chunk(qc)
                project_out_chunk(qc)

        # one output DMA, reading the whole shard tensor: it depends on all
        # 8 RS ops, so the scheduler can only place it at the very end of
        # the sync queue where its RS-completion wait blocks nothing.
        tc.cur_priority += 1_000_000
        nc.sync.dma_start(io["out"], shard_all)


def build(mask_mode="causal", s=S, mm_dtype="bf16", with_bias=True):
    """Build the SPMD Bass module for one core. (mm_dtype is accepted for
    compatibility; the kernel always runs bf16 matmuls / fp32 accum.)"""
    assert mask_mode in ("causal", "zeros", "generic")
    assert s % FD == 0
    nc = bacc.Bacc(
        "TRN2", target_bir_lowering=False, debug=False, num_devices=N_CORES
    )
    NQ = s // FD
    ND = D // P
    io = {}
    for name in ("xq", "xk", "xv"):
        # host passes x^T pre-swizzled: [chunk, partition, d-tile, seq]
        io[name] = nc.dram_tensor(
            name, [NQ, P, ND, FD], BF16, kind="ExternalInput"
        ).ap()
    for name in ("wq", "wk", "wv"):
        io[name] = nc.dram_tensor(
            name, [P, ND, DHH], BF16, kind="ExternalInput"
        ).ap()
    io["wo"] = nc.dram_tensor(
        "wo", [P, DHH // P, D], BF16, kind="ExternalInput"
    ).ap()
    for name in ("bq", "bk", "bv"):
        io[name] = nc.dram_tensor(name, [1, DHH], BF16, kind="ExternalInput").ap()
    if mask_mode == "generic":
        io["maskT"] = nc.dram_tensor(
            "maskT", [s, s], F32, kind="ExternalInput"
        ).ap()
    # output: this core's shard rows (see assemble for the row mapping)
    io["out"] = nc.dram_tensor(
        "out", [s // TP, D], BF16, kind="ExternalOutput"
    ).ap()

    with tile.TileContext(nc) as tc:
        _emit(tc, io, mask_mode, s, with_bias)
    nc.compile()
    return nc


def detect_mask_mode(mask, s=S):
    m = np.asarray(mask).reshape(s, s)
    if not np.any(m):
        return "zeros"
    causal = np.where(
        np.tril(np.ones((s, s), dtype=bool)), 0.0, np.float32(NEG)
    ).astype(np.float32)
    if np.array_equal(m, causal):
        return "causal"
    return "generic"


def make_in_maps(q, k, v, mask, Wq, bq, Wk, bk, Wv, bv, Wo, bo, mask_mode,
                 s=S):
    BF = ml_dtypes.bfloat16
    NQ = s // FD
    ND = D // P
    c32 = lambda a: np.ascontiguousarray(a, dtype=np.float32)

    def swz_x(x):  # [s, D] -> bf16 [NQ, P, ND, FD]: row a*P+p -> [.., p, a, ..]
        xt = np.asarray(x).T.astype(BF)                       # [D, s]
        return np.ascontiguousarray(
            xt.reshape(ND, P, NQ, FD).transpose(2, 1, 0, 3)
        )

    def swz_w(w):  # [D, DHH] -> bf16 [P, ND, DHH]
        return np.ascontiguousarray(
            np.asarray(w, dtype=np.float32).astype(BF)
            .reshape(ND, P, DHH).transpose(1, 0, 2)
        )

    # one host-side transpose/swizzle per (batch, tensor), shared by TP group
    xs = [[swz_x(np.asarray(t)[g]) for t in (q, k, v)] for g in range(DP)]
    in_maps = []
    for c in range(N_CORES):
        g, r = c // TP, c % TP
        sl = slice(r * DHH, (r + 1) * DHH)
        m = {
            "xq": xs[g][0], "xk": xs[g][1], "xv": xs[g][2],
            "wq": swz_w(np.asarray(Wq)[:, sl]),
            "wk": swz_w(np.asarray(Wk)[:, sl]),
            "wv": swz_w(np.asarray(Wv)[:, sl]),
            "wo": np.ascontiguousarray(
                np.asarray(Wo, dtype=np.float32)[sl, :].astype(BF)
                .reshape(DHH // P, P, D).transpose(1, 0, 2)
            ),
            "bq": np.asarray(bq, dtype=np.float32)[sl].astype(BF).reshape(1, DHH),
            "bk": np.asarray(bk, dtype=np.float32)[sl].astype(BF).reshape(1, DHH),
            "bv": np.asarray(bv, dtype=np.float32)[sl].astype(BF).reshape(1, DHH),
        }
        if mask_mode == "generic":
            # pre-scaled by sqrt(DH) so exp((s + m*8)/8) == exp(s/8 + m)
            m["maskT"] = c32(
                np.asarray(mask).reshape(s, s).T * np.float32(DH) ** 0.5
            )
        in_maps.append(m)
    return in_maps


def assemble(results, bo, s=S):
    out = np.empty((B, s, D), np.float32)
    NQ_ = s // FD
    for c in range(N_CORES):
        g, r = c // TP, c % TP
        shard = np.asarray(results[c]["out"]).astype(np.float32)
        for qc in range(NQ_):
            pieces = 2 if qc == NQ_ - 1 else 1
            prow = FD // pieces       # query rows per piece
            rows = prow // TP         # this core's rows per piece
            for p in range(pieces):
                off = (qc * FD + p * prow) // TP
                q0 = qc * FD + p * prow + r * rows
                out[g, q0:q0 + rows, :] = shard[off:off + rows]
    out += np.asarray(bo, dtype=np.float32)[None, None, :]
    return out


_cache = {}
MM_DTYPE = "bf16"  # retained for test.py compatibility; always bf16


def kernel(q, k, v, mask, Wq, bq, Wk, bk, Wv, bv, Wo, bo):
    mask_mode = detect_mask_mode(mask)
    with_bias = any(np.any(np.asarray(b)) for b in (bq, bk, bv))
    key = (mask_mode, with_bias)
    if key not in _cache:
        _cache[key] = build(mask_mode=mask_mode, with_bias=with_bias)
    nc = _cache[key]
    in_maps = make_in_maps(
        q, k, v, mask, Wq, bq, Wk, bk, Wv, bv, Wo, bo, mask_mode
    )
    res = run_bass_kernel_spmd(nc, in_maps, list(range(N_CORES)))
    return assemble(res.results, bo)
